# revision 38
# baseline (speedup 1.0000x reference)
"""CompressKV gating kernel for 8 Trainium2 NeuronCores.

Reference computation (per batch b, head h):
    x_s = x[b, :, h, :]                                  # [N=4096, D=128]
    windows n = 0..254, rows r = 16n + k, k = 0..31
    logits[n, g] = sum_{k,d} x_s[16n+k, d] * W[g, k, d]  # W = W_gate.reshape(32,32,128)
    gate = softmax_g(logits)
    out[n, d] = sum_k gate[n, k] * x_s[16n+k, d]

Sharding: B*H = 32 (b,h) slices, 4 per core, data/tensor parallel, no
cross-core communication.  Host pre-packs x per core in two bf16 layouts:
  xn : row-chunked native    [4, 128(p), 32(c)*128(d)]  (chunk c = rows 128c+p)
  xtk: residue-major d-major [4, 128(d), 16(k')*256(m)] (col 256k'+m = row 16m+k')
plus the gate weight packed d-major with the two k-halves side by side:
  wt2[d, 64k' + 32j + g] = W[g, 16j+k', d].

On-device pipeline per slice:
  A) logits partials via 16 accumulating matmuls, M=64 (both k-halves of a
     residue k' share one contiguous 256-col moving stream), two col-groups
     (k' 0..7 -> psum rows 0:64, k' 8..15 -> rows 64:128) so LDWEIGHTS of one
     group hides under the other group's matmul.
  B) fold 4 row-groups with the j=1 terms shifted one window (DVE),
     exp (ACT) -> e[k=32, n=255] bf16; denominators via ones-matmul ->
     f32 row, shipped to host (normalization happens on host).
  C) banded-matrix pooling: S[r, window] built from e with 9 partition-shift
     band matmuls; 32 matmuls with x chunks stationary -> psum outT[d, n],
     cast to bf16 -> DMA out.  Host transposes + divides by den.
"""

import sys

import numpy as np

for _p in ("/opt/trn_rl_repo", "/opt/pypackages"):
    if _p not in sys.path:
        sys.path.append(_p)

import ml_dtypes

_B, _N, _H, _D = 2, 4096, 16, 128
_K = 32          # window (kernel) size
_ST = 16         # stride
_NB = 255        # num windows
_NC = 8          # cores
_SL = 4          # (b,h) slices per core
_NR = 16         # residues k' per slice
_NM = 256        # m extent (row = 16m + k')

_prog_cache = {}


def _build_program():
    import concourse.mybir as mybir
    from concourse import bacc, tile

    f32 = mybir.dt.float32
    bf16 = mybir.dt.bfloat16

    nc = bacc.Bacc()
    xn = nc.dram_tensor("xn", [_SL, 128, 32 * _D], bf16, kind="ExternalInput")
    xtk = nc.dram_tensor("xtk", [_SL, 128, _NR * _NM], bf16, kind="ExternalInput")
    wt2 = nc.dram_tensor("wt2", [128, _NR * 64], bf16, kind="ExternalInput")
    out = nc.dram_tensor("out", [_SL, 128, _NB], bf16, kind="ExternalOutput")
    den = nc.dram_tensor("den", [1, _SL * _NB], f32, kind="ExternalOutput")

    with tile.TileContext(nc) as tc:
        with (
            tc.tile_pool(name="const", bufs=1) as cpool,
            tc.tile_pool(name="xtk", bufs=4) as xpool,
            tc.tile_pool(name="xn", bufs=4) as npool,
            tc.tile_pool(name="small", bufs=3) as spool,
            tc.tile_pool(name="psQ", bufs=2, space="PSUM") as psq_pool,
            tc.tile_pool(name="psS", bufs=2, space="PSUM") as pss_pool,
            tc.tile_pool(name="psC", bufs=3, space="PSUM") as psc_pool,
            tc.tile_pool(name="psM", bufs=1, space="PSUM") as psm_pool,
        ):
            ones32 = cpool.tile([32, 1], bf16)
            nc.vector.memset(ones32[:], 1.0)
            # band master: mband[k, c] = 1 iff c == k + 128.  Slicing cols
            # [144-16j : 272-16j] gives the [32k, 128r] band matrix with
            # 1 at r == k + 16j - 16 (rows outside [0,128) auto-dropped).
            mband = cpool.tile([32, 272], bf16)
            nc.gpsimd.memset(mband[:], 0.0)
            nc.gpsimd.affine_select(
                out=mband[:],
                in_=mband[:],
                compare_op=mybir.AluOpType.not_equal,
                fill=1.0,
                base=128,
                # fill where 128 + x - y == 0, i.e. y == x + 128
                pattern=[[-1, 272]],
                channel_multiplier=1,
            )
            wt_sb = cpool.tile([128, _NR * 64], bf16)
            nc.sync.dma_start(wt_sb[:], wt2[:, :])
            den_all = cpool.tile([1, _SL * _NB], f32)

            # All input DMAs ride the SP ring (sync engine does no compute,
            # so dispatches are never head-of-line blocked by compute).  They
            # are emitted two slices ahead of the consuming compute so per-tag
            # DMA semaphores have completed by their next use (issuing them
            # all up front serializes the ring on semaphore round-trips).
            # Few, large transfers: the framework recycles a pool of ~10 DMA
            # completion semaphores round-robin across BOTH rings; many small
            # transfers make dispatches wait on old transfers (cross-ring
            # serialization), starving the queue.
            # All xtk transfers go first (stage A needs them early), then all
            # xn transfers (stage C consumes them late).  This also tells the
            # tile scheduler that every A_{s+1} is data-ready early, so it
            # schedules A_{s+1} ahead of bands_s/C_s on the PE instead of
            # serializing the fold->bands->pool->A dependency ring.
            slice_dma = {}
            for s in range(_SL):
                xtk_t = xpool.tile([128, 4096], bf16, tag="xtk", name=f"xtk_{s}")
                if s == 0:
                    # split only the first slice so stage A can start earlier
                    nc.sync.dma_start(xtk_t[:, 0:2048], xtk[s, :, 0:2048])
                    nc.sync.dma_start(xtk_t[:, 2048:4096], xtk[s, :, 2048:4096])
                else:
                    nc.sync.dma_start(xtk_t[:], xtk[s, :, :])
                slice_dma[s] = xtk_t
            slice_xn = {}
            for s in range(_SL):
                xn_t = npool.tile([128, 4096], bf16, tag="xn", name=f"xn_{s}")
                nc.sync.dma_start(xn_t[:], xn[s, :, :])
                slice_xn[s] = xn_t
            slice_a = {}

            def emit_a(s):
                # ---- stage A ----
                # column block p holds residue k'_p = (p//2) + 8*(p%2), so the
                # two PE column groups (even p -> psum rows 0:64, odd p ->
                # rows 64:128) consume the DMA stream in arrival order and
                # their streams overlap in the PE array.
                xtk_t = slice_dma.pop(s)
                psQ = psq_pool.tile([128, _NM], f32, tag="psQ", name=f"psQ_{s}")
                for p in range(16):
                    g = p % 2
                    nc.tensor.matmul(
                        psQ[64 * g : 64 * g + 64, :],
                        wt_sb[:, 64 * p : 64 * p + 64],
                        xtk_t[:, 256 * p : 256 * p + 256],
                        start=(p < 2),
                        stop=(p >= 14),
                        tile_position=(0, 64 * g),
                        skip_group_check=True,
                    )
                slice_a[s] = psQ

            emit_a(0)
            for s in range(_SL):
                # software pipeline: emit the next slice's stage A before this
                # slice's B/C chain so the PE streams A_{s+1} during the
                # fold/exp latency instead of idling in-order behind bands_s
                if s + 1 < _SL:
                    emit_a(s + 1)
                psQ = slice_a.pop(s)
                xn_t = slice_xn.pop(s)

                # fold: logits[g, n] = sum over both col-groups of
                #   Q[(g,j=0), n] + Q[(g,j=1), n+1]
                # (DVE: one PSUM operand per op; SBUF operands must share
                # their base partition -- so chain the psum adds)
                f0 = spool.tile([32, _NB], f32, tag="f0")
                f1 = spool.tile([32, _NB], f32, tag="f1")
                f2 = spool.tile([32, _NB], f32, tag="f2")
                logits = spool.tile([32, _NB], f32, tag="logits")
                nc.vector.tensor_copy(f0[:], psQ[32:64, 1:256])
                nc.vector.tensor_add(f1[:], f0[:], psQ[96:128, 1:256])
                nc.vector.tensor_add(f2[:], f1[:], psQ[0:32, 0:255])
                nc.vector.tensor_add(logits[:], f2[:], psQ[64:96, 0:255])

                # exp (denominators are computed after stage C, off the
                # critical path: C needs only S, not den)
                e_kn = spool.tile([32, _NB], bf16, tag="e_kn")
                nc.scalar.activation(
                    e_kn[:], logits[:], mybir.ActivationFunctionType.Exp
                )

                # S matrix (class-major cols 32j + c): window n = 8c-1+j,
                # S[r=16j-16+k, 32j+c] = e[k, n].  Built on PE via band-matrix
                # lhsT (partition placement encoded in the matrix), since
                # engine partition bases must be 32-aligned.
                psS = pss_pool.tile([128, 9 * 32], f32, tag="psS")
                for j in range(9):
                    c0 = 1 if j == 0 else 0
                    c1 = 31 if j == 8 else 32
                    nc.tensor.matmul(
                        psS[:, 32 * j + c0 : 32 * j + c1],
                        mband[:, 144 - 16 * j : 272 - 16 * j],
                        e_kn[:, 8 * c0 + j - 1 : 8 * (c1 - 1) + j : 8],
                        start=True,
                        stop=True,
                        skip_group_check=True,
                    )
                S_sb = spool.tile([128, 9 * 32], bf16, tag="S")
                # cols 0 and 287 are never written (invalid windows) nor read
                nc.scalar.activation(
                    S_sb[:, 1:287], psS[:, 1:287],
                    mybir.ActivationFunctionType.Copy,
                )

                # ---- stage C: pooled outT[d, n] (unnormalized) ----
                psC = psc_pool.tile([128, _NB], f32, tag="psC")
                nc.vector.memset(psC[:], 0.0)
                for c in range(32):
                    j0 = 1 if c == 0 else 0
                    j1 = 8 if c == 31 else 9
                    xn_chunk = xn_t[:, 128 * c : 128 * c + 128]
                    nc.tensor.matmul(
                        psC[:, 8 * c - 1 + j0 : 8 * c - 1 + j1],
                        xn_chunk,
                        S_sb[:, 32 * j0 + c : 32 * (j1 - 1) + c + 1 : 32],
                        start=False,
                        stop=(c == 31),
                        skip_group_check=True,
                    )

                o_sb = spool.tile([128, _NB], bf16, tag="o")
                nc.scalar.activation(
                    o_sb[:], psC[:], mybir.ActivationFunctionType.Copy
                )
                nc.scalar.dma_start(out[s, :, :], o_sb[:])

                # denominators (normalization happens on host)
                psM = psm_pool.tile([1, _NB], f32, tag="psM")
                nc.tensor.matmul(psM[0:1, :], ones32[:, 0:1], e_kn[:, :])
                nc.vector.tensor_copy(
                    den_all[:, _NB * s : _NB * s + _NB], psM[0:1, :]
                )
            nc.scalar.dma_start(den[:, :], den_all[:])

    nc.compile()
    return nc


def _get_program():
    if "nc" not in _prog_cache:
        _prog_cache["nc"] = _build_program()
    return _prog_cache["nc"]


def _host_inputs(x, W_gate):
    bf16 = ml_dtypes.bfloat16
    x = np.asarray(x, dtype=np.float32)
    W = np.asarray(W_gate, dtype=np.float32)
    # column block p holds residue k'_p = (p//2) + 8*(p%2), interleaving the
    # two PE column groups so the DMA stream feeds them alternately
    perm = [(p // 2) + 8 * (p % 2) for p in range(16)]
    # wt2[d, 64p + 32j + g] = W_gate[g, (16j+k'_p)*128 + d]
    W4 = W.reshape(_K, 2, _NR, _D)  # [g, j, k', d]
    wt2_host = np.ascontiguousarray(
        W4.transpose(3, 2, 1, 0)[:, perm].reshape(_D, _NR * 64)
    ).astype(bf16)
    in_maps = []
    for core in range(_NC):
        xn = np.empty((_SL, 128, 32 * _D), dtype=bf16)
        xtk = np.empty((_SL, 128, _NR * _NM), dtype=bf16)
        for si in range(_SL):
            p = core * _SL + si
            b, h = p // _H, p % _H
            xs = x[b, :, h, :]  # [4096, 128]
            xn[si] = (
                xs.reshape(32, 128, _D).transpose(1, 0, 2).reshape(128, 32 * _D)
            ).astype(bf16)
            # xtk[d, 256p + m] = xs[16m + k'_p, d]
            xtk[si] = (
                xs.reshape(_NM, _NR, _D).transpose(2, 1, 0)[:, perm]
                .reshape(128, _NR * _NM)
            ).astype(bf16)
        in_maps.append({"xn": xn, "xtk": xtk, "wt2": wt2_host})
    return in_maps


def _assemble(results):
    out = np.empty((_B, _NB, _H, _D), dtype=np.float32)
    for core in range(_NC):
        o = np.asarray(results[core]["out"], dtype=np.float32)  # [SL, 128, 255]
        dn = np.asarray(results[core]["den"], dtype=np.float32).reshape(_SL, _NB)
        for si in range(_SL):
            p = core * _SL + si
            out[p // _H, :, p % _H, :] = o[si].T / dn[si][:, None]
    return out


def _install_trace_hooks():
    """Shim the axon NTFF profile hook (missing in this image) so
    run_bass_kernel_spmd(trace=True) can collect a HW profile, and neuter
    the artifact upload (zero-egress container)."""
    import contextlib
    import ctypes
    import types

    try:
        from antenv.axon_hooks import get_axon_ntff_profile_hook  # noqa: F401

        return
    except ImportError:
        pass

    lib = ctypes.CDLL("/opt/axon/libaxon_pjrt.so")
    if not hasattr(lib, "axon_start_nrt_profile"):
        return
    lib.axon_start_nrt_profile.argtypes = [
        ctypes.POINTER(ctypes.c_int64),
        ctypes.c_size_t,
    ]
    lib.axon_start_nrt_profile.restype = ctypes.c_int64
    lib.axon_stop_nrt_profile.argtypes = [ctypes.c_char_p]
    lib.axon_stop_nrt_profile.restype = ctypes.c_int64

    @contextlib.contextmanager
    def _hook(output_dir, device_ids):
        import jax

        jax.devices()
        if device_ids:
            ids = (ctypes.c_int64 * len(device_ids))(*device_ids)
            rc = lib.axon_start_nrt_profile(ids, len(device_ids))
        else:
            rc = lib.axon_start_nrt_profile(None, 0)
        if rc != 0:
            raise RuntimeError(f"axon_start_nrt_profile rc={rc}")
        try:
            yield
        finally:
            n = lib.axon_stop_nrt_profile(str(output_dir).encode())
            print(f"profile: {n} file(s) written to {output_dir}")

    mod = types.ModuleType("antenv.axon_hooks")
    mod.get_axon_ntff_profile_hook = lambda: _hook
    mod.set_axon_ntff_profile_hook = lambda h: None
    sys.modules["antenv.axon_hooks"] = mod

    from concourse import bass_utils as bu

    bu.upload_artifacts = lambda tmpdir: tmpdir


def run(x, W_gate, trace=False, **kw):
    from concourse.bass_utils import run_bass_kernel_spmd

    if trace:
        _install_trace_hooks()
    nc = _get_program()
    in_maps = _host_inputs(x, W_gate)
    res = run_bass_kernel_spmd(nc, in_maps, list(range(_NC)), trace=trace, **kw)
    return _assemble(res.results), res


def kernel(x, W_gate):
    out, _ = run(x, W_gate)
    return out


# revision 40
# speedup vs baseline: 1.0035x; 1.0035x over previous
"""CompressKV gating kernel for 8 Trainium2 NeuronCores.

Reference computation (per batch b, head h):
    x_s = x[b, :, h, :]                                  # [N=4096, D=128]
    windows n = 0..254, rows r = 16n + k, k = 0..31
    logits[n, g] = sum_{k,d} x_s[16n+k, d] * W[g, k, d]  # W = W_gate.reshape(32,32,128)
    gate = softmax_g(logits)
    out[n, d] = sum_k gate[n, k] * x_s[16n+k, d]

Sharding: B*H = 32 (b,h) slices, 4 per core, data/tensor parallel, no
cross-core communication.  Host pre-packs x per core in two bf16 layouts:
  xn : row-chunked native    [4, 128(p), 32(c)*128(d)]  (chunk c = rows 128c+p)
  xtk: residue-major d-major [4, 128(d), 16(k')*256(m)] (col 256k'+m = row 16m+k')
plus the gate weight packed d-major with the two k-halves side by side:
  wt2[d, 64k' + 32j + g] = W[g, 16j+k', d].

On-device pipeline per slice:
  A) logits partials via 16 accumulating matmuls, M=64 (both k-halves of a
     residue k' share one contiguous 256-col moving stream), two col-groups
     (k' 0..7 -> psum rows 0:64, k' 8..15 -> rows 64:128) so LDWEIGHTS of one
     group hides under the other group's matmul.
  B) fold 4 row-groups with the j=1 terms shifted one window (DVE),
     exp (ACT) -> e[k=32, n=255] bf16; denominators via ones-matmul ->
     f32 row, shipped to host (normalization happens on host).
  C) banded-matrix pooling: S[r, window] built from e with 9 partition-shift
     band matmuls; 32 matmuls with x chunks stationary -> psum outT[d, n],
     cast to bf16 -> DMA out.  Host transposes + divides by den.
"""

import sys

import numpy as np

for _p in ("/opt/trn_rl_repo", "/opt/pypackages"):
    if _p not in sys.path:
        sys.path.append(_p)

import ml_dtypes

_B, _N, _H, _D = 2, 4096, 16, 128
_K = 32          # window (kernel) size
_ST = 16         # stride
_NB = 255        # num windows
_NC = 8          # cores
_SL = 4          # (b,h) slices per core
_NR = 16         # residues k' per slice
_NM = 256        # m extent (row = 16m + k')

_prog_cache = {}


def _build_program():
    import concourse.mybir as mybir
    from concourse import bacc, tile

    f32 = mybir.dt.float32
    bf16 = mybir.dt.bfloat16

    nc = bacc.Bacc()
    xn = nc.dram_tensor("xn", [_SL, 128, 32 * _D], bf16, kind="ExternalInput")
    xtk = nc.dram_tensor("xtk", [_SL, 128, _NR * _NM], bf16, kind="ExternalInput")
    wt2 = nc.dram_tensor("wt2", [128, _NR * 64], bf16, kind="ExternalInput")
    out = nc.dram_tensor("out", [_SL, 128, _NB], bf16, kind="ExternalOutput")
    den = nc.dram_tensor("den", [1, _SL * _NB], f32, kind="ExternalOutput")

    with tile.TileContext(nc) as tc:
        with (
            tc.tile_pool(name="const", bufs=1) as cpool,
            tc.tile_pool(name="xtk", bufs=4) as xpool,
            tc.tile_pool(name="xn", bufs=4) as npool,
            tc.tile_pool(name="small", bufs=3) as spool,
            tc.tile_pool(name="psQ", bufs=2, space="PSUM") as psq_pool,
            tc.tile_pool(name="psS", bufs=2, space="PSUM") as pss_pool,
            tc.tile_pool(name="psC", bufs=2, space="PSUM") as psc_pool,
            tc.tile_pool(name="psM", bufs=1, space="PSUM") as psm_pool,
        ):
            ones32 = cpool.tile([32, 1], bf16)
            nc.vector.memset(ones32[:], 1.0)
            # band master: mband[k, c] = 1 iff c == k + 128.  Slicing cols
            # [144-16j : 272-16j] gives the [32k, 128r] band matrix with
            # 1 at r == k + 16j - 16 (rows outside [0,128) auto-dropped).
            mband = cpool.tile([32, 272], bf16)
            nc.gpsimd.memset(mband[:], 0.0)
            nc.gpsimd.affine_select(
                out=mband[:],
                in_=mband[:],
                compare_op=mybir.AluOpType.not_equal,
                fill=1.0,
                base=128,
                # fill where 128 + x - y == 0, i.e. y == x + 128
                pattern=[[-1, 272]],
                channel_multiplier=1,
            )
            wt_sb = cpool.tile([128, _NR * 64], bf16)
            nc.sync.dma_start(wt_sb[:], wt2[:, :])
            den_all = cpool.tile([1, _SL * _NB], f32)

            # All input DMAs ride the SP ring (sync engine does no compute,
            # so dispatches are never head-of-line blocked by compute).  They
            # are emitted two slices ahead of the consuming compute so per-tag
            # DMA semaphores have completed by their next use (issuing them
            # all up front serializes the ring on semaphore round-trips).
            # Few, large transfers: the framework recycles a pool of ~10 DMA
            # completion semaphores round-robin across BOTH rings; many small
            # transfers make dispatches wait on old transfers (cross-ring
            # serialization), starving the queue.
            # All xtk transfers go first (stage A needs them early), then all
            # xn transfers (stage C consumes them late).  This also tells the
            # tile scheduler that every A_{s+1} is data-ready early, so it
            # schedules A_{s+1} ahead of bands_s/C_s on the PE instead of
            # serializing the fold->bands->pool->A dependency ring.
            slice_dma = {}
            for s in range(_SL):
                xtk_t = xpool.tile([128, 4096], bf16, tag="xtk", name=f"xtk_{s}")
                if s == 0:
                    # split only the first slice so stage A can start earlier
                    nc.sync.dma_start(xtk_t[:, 0:2048], xtk[s, :, 0:2048])
                    nc.sync.dma_start(xtk_t[:, 2048:4096], xtk[s, :, 2048:4096])
                else:
                    nc.sync.dma_start(xtk_t[:], xtk[s, :, :])
                slice_dma[s] = xtk_t
            slice_xn = {}
            for s in range(_SL):
                xn_t = npool.tile([128, 4096], bf16, tag="xn", name=f"xn_{s}")
                nc.sync.dma_start(xn_t[:], xn[s, :, :])
                slice_xn[s] = xn_t
            slice_a = {}

            def emit_a(s):
                # ---- stage A ----
                # column block p holds residue k'_p = (p//2) + 8*(p%2), so the
                # two PE column groups (even p -> psum rows 0:64, odd p ->
                # rows 64:128) consume the DMA stream in arrival order and
                # their streams overlap in the PE array.
                xtk_t = slice_dma.pop(s)
                psQ = psq_pool.tile([128, _NM], f32, tag="psQ", name=f"psQ_{s}")
                for p in range(16):
                    g = p % 2
                    nc.tensor.matmul(
                        psQ[64 * g : 64 * g + 64, :],
                        wt_sb[:, 64 * p : 64 * p + 64],
                        xtk_t[:, 256 * p : 256 * p + 256],
                        start=(p < 2),
                        stop=(p >= 14),
                        tile_position=(0, 64 * g),
                        skip_group_check=True,
                    )
                slice_a[s] = psQ

            emit_a(0)
            for s in range(_SL):
                # software pipeline: emit the next slice's stage A before this
                # slice's B/C chain so the PE streams A_{s+1} during the
                # fold/exp latency instead of idling in-order behind bands_s
                if s + 1 < _SL:
                    emit_a(s + 1)
                psQ = slice_a.pop(s)
                xn_t = slice_xn.pop(s)

                # fold: logits[g, n] = sum over both col-groups of
                #   Q[(g,j=0), n] + Q[(g,j=1), n+1]
                # (DVE: one PSUM operand per op; SBUF operands must share
                # their base partition -- so chain the psum adds)
                f0 = spool.tile([32, _NB], f32, tag="f0")
                f1 = spool.tile([32, _NB], f32, tag="f1")
                f2 = spool.tile([32, _NB], f32, tag="f2")
                logits = spool.tile([32, _NB], f32, tag="logits")
                nc.vector.tensor_copy(f0[:], psQ[32:64, 1:256])
                nc.vector.tensor_add(f1[:], f0[:], psQ[96:128, 1:256])
                nc.vector.tensor_add(f2[:], f1[:], psQ[0:32, 0:255])
                nc.vector.tensor_add(logits[:], f2[:], psQ[64:96, 0:255])

                # exp (denominators are computed after stage C, off the
                # critical path: C needs only S, not den)
                e_kn = spool.tile([32, _NB], bf16, tag="e_kn")
                nc.scalar.activation(
                    e_kn[:], logits[:], mybir.ActivationFunctionType.Exp
                )

                # S matrix (class-major cols 32j + c): window n = 8c-1+j,
                # S[r=16j-16+k, 32j+c] = e[k, n].  Built on PE via band-matrix
                # lhsT (partition placement encoded in the matrix), since
                # engine partition bases must be 32-aligned.
                psS = pss_pool.tile([128, 9 * 32], f32, tag="psS")
                for j in range(9):
                    c0 = 1 if j == 0 else 0
                    c1 = 31 if j == 8 else 32
                    nc.tensor.matmul(
                        psS[:, 32 * j + c0 : 32 * j + c1],
                        mband[:, 144 - 16 * j : 272 - 16 * j],
                        e_kn[:, 8 * c0 + j - 1 : 8 * (c1 - 1) + j : 8],
                        start=True,
                        stop=True,
                        skip_group_check=True,
                    )
                S_sb = spool.tile([128, 9 * 32], bf16, tag="S")
                # cols 0 and 287 are never written (invalid windows) nor read
                nc.scalar.activation(
                    S_sb[:, 1:287], psS[:, 1:287],
                    mybir.ActivationFunctionType.Copy,
                )

                # ---- stage C: pooled outT[d, n] (unnormalized) ----
                psC = psc_pool.tile([128, _NB], f32, tag="psC")
                nc.vector.memset(psC[:], 0.0)
                for c in range(32):
                    j0 = 1 if c == 0 else 0
                    j1 = 8 if c == 31 else 9
                    xn_chunk = xn_t[:, 128 * c : 128 * c + 128]
                    nc.tensor.matmul(
                        psC[:, 8 * c - 1 + j0 : 8 * c - 1 + j1],
                        xn_chunk,
                        S_sb[:, 32 * j0 + c : 32 * (j1 - 1) + c + 1 : 32],
                        start=False,
                        stop=(c == 31),
                        skip_group_check=True,
                    )

                o_sb = spool.tile([128, _NB], bf16, tag="o")
                nc.scalar.activation(
                    o_sb[:], psC[:], mybir.ActivationFunctionType.Copy
                )
                nc.scalar.dma_start(out[s, :, :], o_sb[:])

                # denominators (normalization happens on host).  The psum
                # copy rides ACT, not DVE: on DVE it waits for the ones
                # matmul (which runs after stage C) and head-of-line blocks
                # the NEXT slice's fold in the in-order DVE queue.
                psM = psm_pool.tile([1, _NB], f32, tag="psM")
                nc.tensor.matmul(psM[0:1, :], ones32[:, 0:1], e_kn[:, :])
                nc.scalar.activation(
                    den_all[:, _NB * s : _NB * s + _NB], psM[0:1, :],
                    mybir.ActivationFunctionType.Copy,
                )
            nc.scalar.dma_start(den[:, :], den_all[:])

    nc.compile()
    return nc


def _get_program():
    if "nc" not in _prog_cache:
        _prog_cache["nc"] = _build_program()
    return _prog_cache["nc"]


def _host_inputs(x, W_gate):
    bf16 = ml_dtypes.bfloat16
    x = np.asarray(x, dtype=np.float32)
    W = np.asarray(W_gate, dtype=np.float32)
    # column block p holds residue k'_p = (p//2) + 8*(p%2), interleaving the
    # two PE column groups so the DMA stream feeds them alternately
    perm = [(p // 2) + 8 * (p % 2) for p in range(16)]
    # wt2[d, 64p + 32j + g] = W_gate[g, (16j+k'_p)*128 + d]
    W4 = W.reshape(_K, 2, _NR, _D)  # [g, j, k', d]
    wt2_host = np.ascontiguousarray(
        W4.transpose(3, 2, 1, 0)[:, perm].reshape(_D, _NR * 64)
    ).astype(bf16)
    in_maps = []
    for core in range(_NC):
        xn = np.empty((_SL, 128, 32 * _D), dtype=bf16)
        xtk = np.empty((_SL, 128, _NR * _NM), dtype=bf16)
        for si in range(_SL):
            p = core * _SL + si
            b, h = p // _H, p % _H
            xs = x[b, :, h, :]  # [4096, 128]
            xn[si] = (
                xs.reshape(32, 128, _D).transpose(1, 0, 2).reshape(128, 32 * _D)
            ).astype(bf16)
            # xtk[d, 256p + m] = xs[16m + k'_p, d]
            xtk[si] = (
                xs.reshape(_NM, _NR, _D).transpose(2, 1, 0)[:, perm]
                .reshape(128, _NR * _NM)
            ).astype(bf16)
        in_maps.append({"xn": xn, "xtk": xtk, "wt2": wt2_host})
    return in_maps


def _assemble(results):
    out = np.empty((_B, _NB, _H, _D), dtype=np.float32)
    for core in range(_NC):
        o = np.asarray(results[core]["out"], dtype=np.float32)  # [SL, 128, 255]
        dn = np.asarray(results[core]["den"], dtype=np.float32).reshape(_SL, _NB)
        for si in range(_SL):
            p = core * _SL + si
            out[p // _H, :, p % _H, :] = o[si].T / dn[si][:, None]
    return out


def _install_trace_hooks():
    """Shim the axon NTFF profile hook (missing in this image) so
    run_bass_kernel_spmd(trace=True) can collect a HW profile, and neuter
    the artifact upload (zero-egress container)."""
    import contextlib
    import ctypes
    import types

    try:
        from antenv.axon_hooks import get_axon_ntff_profile_hook  # noqa: F401

        return
    except ImportError:
        pass

    lib = ctypes.CDLL("/opt/axon/libaxon_pjrt.so")
    if not hasattr(lib, "axon_start_nrt_profile"):
        return
    lib.axon_start_nrt_profile.argtypes = [
        ctypes.POINTER(ctypes.c_int64),
        ctypes.c_size_t,
    ]
    lib.axon_start_nrt_profile.restype = ctypes.c_int64
    lib.axon_stop_nrt_profile.argtypes = [ctypes.c_char_p]
    lib.axon_stop_nrt_profile.restype = ctypes.c_int64

    @contextlib.contextmanager
    def _hook(output_dir, device_ids):
        import jax

        jax.devices()
        if device_ids:
            ids = (ctypes.c_int64 * len(device_ids))(*device_ids)
            rc = lib.axon_start_nrt_profile(ids, len(device_ids))
        else:
            rc = lib.axon_start_nrt_profile(None, 0)
        if rc != 0:
            raise RuntimeError(f"axon_start_nrt_profile rc={rc}")
        try:
            yield
        finally:
            n = lib.axon_stop_nrt_profile(str(output_dir).encode())
            print(f"profile: {n} file(s) written to {output_dir}")

    mod = types.ModuleType("antenv.axon_hooks")
    mod.get_axon_ntff_profile_hook = lambda: _hook
    mod.set_axon_ntff_profile_hook = lambda h: None
    sys.modules["antenv.axon_hooks"] = mod

    from concourse import bass_utils as bu

    bu.upload_artifacts = lambda tmpdir: tmpdir


def run(x, W_gate, trace=False, **kw):
    from concourse.bass_utils import run_bass_kernel_spmd

    if trace:
        _install_trace_hooks()
    nc = _get_program()
    in_maps = _host_inputs(x, W_gate)
    res = run_bass_kernel_spmd(nc, in_maps, list(range(_NC)), trace=trace, **kw)
    return _assemble(res.results), res


def kernel(x, W_gate):
    out, _ = run(x, W_gate)
    return out


# revision 43
# speedup vs baseline: 1.0143x; 1.0108x over previous
"""CompressKV gating kernel for 8 Trainium2 NeuronCores.

Reference computation (per batch b, head h):
    x_s = x[b, :, h, :]                                  # [N=4096, D=128]
    windows n = 0..254, rows r = 16n + k, k = 0..31
    logits[n, g] = sum_{k,d} x_s[16n+k, d] * W[g, k, d]  # W = W_gate.reshape(32,32,128)
    gate = softmax_g(logits)
    out[n, d] = sum_k gate[n, k] * x_s[16n+k, d]

Sharding: B*H = 32 (b,h) slices, 4 per core, data/tensor parallel, no
cross-core communication.  Host pre-packs x per core in two bf16 layouts:
  xn : row-chunked native    [4, 128(p), 32(c)*128(d)]  (chunk c = rows 128c+p)
  xtk: residue-major d-major [4, 128(d), 16(k')*256(m)] (col 256k'+m = row 16m+k')
plus the gate weight packed d-major with the two k-halves side by side:
  wt2[d, 64k' + 32j + g] = W[g, 16j+k', d].

On-device pipeline per slice:
  A) logits partials via 16 accumulating matmuls, M=64 (both k-halves of a
     residue k' share one contiguous 256-col moving stream), two col-groups
     (k' 0..7 -> psum rows 0:64, k' 8..15 -> rows 64:128) so LDWEIGHTS of one
     group hides under the other group's matmul.
  B) fold 4 row-groups with the j=1 terms shifted one window (DVE),
     exp (ACT) -> e[k=32, n=255] bf16; denominators via ones-matmul ->
     f32 row, shipped to host (normalization happens on host).
  C) banded-matrix pooling: S[r, window] built from e with 9 partition-shift
     band matmuls; 32 matmuls with x chunks stationary -> psum outT[d, n],
     cast to bf16 -> DMA out.  Host transposes + divides by den.
"""

import sys

import numpy as np

for _p in ("/opt/trn_rl_repo", "/opt/pypackages"):
    if _p not in sys.path:
        sys.path.append(_p)

import ml_dtypes

_B, _N, _H, _D = 2, 4096, 16, 128
_K = 32          # window (kernel) size
_ST = 16         # stride
_NB = 255        # num windows
_NC = 8          # cores
_SL = 4          # (b,h) slices per core
_NR = 16         # residues k' per slice
_NM = 256        # m extent (row = 16m + k')

_prog_cache = {}


def _build_program():
    import concourse.mybir as mybir
    from concourse import bacc, tile

    f32 = mybir.dt.float32
    bf16 = mybir.dt.bfloat16

    nc = bacc.Bacc()
    xn = nc.dram_tensor("xn", [_SL, 128, 32 * _D], bf16, kind="ExternalInput")
    xtk = nc.dram_tensor("xtk", [_SL, 128, _NR * _NM], bf16, kind="ExternalInput")
    wt2 = nc.dram_tensor("wt2", [128, _NR * 64], bf16, kind="ExternalInput")
    out = nc.dram_tensor("out", [_SL, 128, _NB], bf16, kind="ExternalOutput")
    den = nc.dram_tensor("den", [1, _SL * _NB], f32, kind="ExternalOutput")

    with tile.TileContext(nc) as tc:
        with (
            tc.tile_pool(name="const", bufs=1) as cpool,
            tc.tile_pool(name="xtk", bufs=4) as xpool,
            tc.tile_pool(name="xn", bufs=4) as npool,
            tc.tile_pool(name="small", bufs=3) as spool,
            tc.tile_pool(name="psQ", bufs=2, space="PSUM") as psq_pool,
            tc.tile_pool(name="psS", bufs=2, space="PSUM") as pss_pool,
            tc.tile_pool(name="psC", bufs=2, space="PSUM") as psc_pool,
            tc.tile_pool(name="psM", bufs=1, space="PSUM") as psm_pool,
        ):
            ones32 = cpool.tile([32, 1], bf16)
            nc.vector.memset(ones32[:], 1.0)
            # band master: mband[k, c] = 1 iff c == k + 128.  Slicing cols
            # [144-16j : 272-16j] gives the [32k, 128r] band matrix with
            # 1 at r == k + 16j - 16 (rows outside [0,128) auto-dropped).
            mband = cpool.tile([32, 272], bf16)
            nc.gpsimd.memset(mband[:], 0.0)
            nc.gpsimd.affine_select(
                out=mband[:],
                in_=mband[:],
                compare_op=mybir.AluOpType.not_equal,
                fill=1.0,
                base=128,
                # fill where 128 + x - y == 0, i.e. y == x + 128
                pattern=[[-1, 272]],
                channel_multiplier=1,
            )
            wt_sb = cpool.tile([128, _NR * 64], bf16)
            nc.sync.dma_start(wt_sb[:], wt2[:, :])
            den_all = cpool.tile([1, _SL * _NB], f32)

            # All input DMAs ride the SP ring (sync engine does no compute,
            # so dispatches are never head-of-line blocked by compute).  They
            # are emitted two slices ahead of the consuming compute so per-tag
            # DMA semaphores have completed by their next use (issuing them
            # all up front serializes the ring on semaphore round-trips).
            # Few, large transfers: the framework recycles a pool of ~10 DMA
            # completion semaphores round-robin across BOTH rings; many small
            # transfers make dispatches wait on old transfers (cross-ring
            # serialization), starving the queue.
            # All xtk transfers go first (stage A needs them early), then all
            # xn transfers (stage C consumes them late).  This also tells the
            # tile scheduler that every A_{s+1} is data-ready early, so it
            # schedules A_{s+1} ahead of bands_s/C_s on the PE instead of
            # serializing the fold->bands->pool->A dependency ring.
            slice_dma = {}
            for s in range(_SL):
                xtk_t = xpool.tile([128, 4096], bf16, tag="xtk", name=f"xtk_{s}")
                if s == 0:
                    # split only the first slice so stage A can start earlier
                    nc.sync.dma_start(xtk_t[:, 0:2048], xtk[s, :, 0:2048])
                    nc.sync.dma_start(xtk_t[:, 2048:4096], xtk[s, :, 2048:4096])
                else:
                    nc.sync.dma_start(xtk_t[:], xtk[s, :, :])
                slice_dma[s] = xtk_t
            slice_xn = {}
            for s in range(_SL):
                xn_h = [
                    npool.tile([128, 2048], bf16, tag=f"xn{h}", name=f"xn{h}_{s}")
                    for h in range(2)
                ]
                nc.sync.dma_start(xn_h[0][:], xn[s, :, 0:2048])
                nc.sync.dma_start(xn_h[1][:], xn[s, :, 2048:4096])
                slice_xn[s] = xn_h
            slice_a = {}

            def emit_a(s):
                # ---- stage A ----
                # column block p holds residue k'_p = (p//2) + 8*(p%2), so the
                # two PE column groups (even p -> psum rows 0:64, odd p ->
                # rows 64:128) consume the DMA stream in arrival order and
                # their streams overlap in the PE array.
                xtk_t = slice_dma.pop(s)
                psQ = psq_pool.tile([128, _NM], f32, tag="psQ", name=f"psQ_{s}")
                for p in range(16):
                    g = p % 2
                    nc.tensor.matmul(
                        psQ[64 * g : 64 * g + 64, :],
                        wt_sb[:, 64 * p : 64 * p + 64],
                        xtk_t[:, 256 * p : 256 * p + 256],
                        start=(p < 2),
                        stop=(p >= 14),
                        tile_position=(0, 64 * g),
                        skip_group_check=True,
                    )
                slice_a[s] = psQ

            emit_a(0)
            for s in range(_SL):
                # software pipeline: emit the next slice's stage A before this
                # slice's B/C chain so the PE streams A_{s+1} during the
                # fold/exp latency instead of idling in-order behind bands_s
                if s + 1 < _SL:
                    emit_a(s + 1)
                psQ = slice_a.pop(s)
                xn_h = slice_xn.pop(s)

                # fold: logits[g, n] = sum over both col-groups of
                #   Q[(g,j=0), n] + Q[(g,j=1), n+1]
                # (DVE: one PSUM operand per op; SBUF operands must share
                # their base partition -- so chain the psum adds)
                f0 = spool.tile([32, _NB], f32, tag="f0")
                f1 = spool.tile([32, _NB], f32, tag="f1")
                f2 = spool.tile([32, _NB], f32, tag="f2")
                logits = spool.tile([32, _NB], f32, tag="logits")
                nc.vector.tensor_copy(f0[:], psQ[32:64, 1:256])
                nc.vector.tensor_add(f1[:], f0[:], psQ[96:128, 1:256])
                nc.vector.tensor_add(f2[:], f1[:], psQ[0:32, 0:255])
                nc.vector.tensor_add(logits[:], f2[:], psQ[64:96, 0:255])

                # exp (denominators are computed after stage C, off the
                # critical path: C needs only S, not den)
                e_kn = spool.tile([32, _NB], bf16, tag="e_kn")
                nc.scalar.activation(
                    e_kn[:], logits[:], mybir.ActivationFunctionType.Exp
                )

                # S matrix (class-major cols 32j + c): window n = 8c-1+j,
                # S[r=16j-16+k, 32j+c] = e[k, n].  Built on PE via band-matrix
                # lhsT (partition placement encoded in the matrix), since
                # engine partition bases must be 32-aligned.
                psS = pss_pool.tile([128, 9 * 32], f32, tag="psS")
                for j in range(9):
                    c0 = 1 if j == 0 else 0
                    c1 = 31 if j == 8 else 32
                    nc.tensor.matmul(
                        psS[:, 32 * j + c0 : 32 * j + c1],
                        mband[:, 144 - 16 * j : 272 - 16 * j],
                        e_kn[:, 8 * c0 + j - 1 : 8 * (c1 - 1) + j : 8],
                        start=True,
                        stop=True,
                        skip_group_check=True,
                    )
                S_sb = spool.tile([128, 9 * 32], bf16, tag="S")
                # cols 0 and 287 are never written (invalid windows) nor read
                nc.scalar.activation(
                    S_sb[:, 1:287], psS[:, 1:287],
                    mybir.ActivationFunctionType.Copy,
                )

                # ---- stage C: pooled outT[d, n] (unnormalized), split into
                # xn halves so windows 0..126 (final after chunk 15) cast and
                # ship while the second xn half is still streaming in ----
                psC = psc_pool.tile([128, _NB], f32, tag="psC")
                nc.vector.memset(psC[:], 0.0)
                o_sb = spool.tile([128, _NB], bf16, tag="o")
                for h in range(2):
                    for c in range(16 * h, 16 * h + 16):
                        j0 = 1 if c == 0 else 0
                        j1 = 8 if c == 31 else 9
                        xn_chunk = xn_h[h][:, 128 * (c % 16) : 128 * (c % 16) + 128]
                        nc.tensor.matmul(
                            psC[:, 8 * c - 1 + j0 : 8 * c - 1 + j1],
                            xn_chunk,
                            S_sb[:, 32 * j0 + c : 32 * (j1 - 1) + c + 1 : 32],
                            start=False,
                            stop=(c == 31),
                            skip_group_check=True,
                        )
                    # window 127 spans both xn halves (chunks 15 and 16)
                    o0, o1 = (0, 127) if h == 0 else (127, 255)
                    nc.scalar.activation(
                        o_sb[:, o0:o1], psC[:, o0:o1],
                        mybir.ActivationFunctionType.Copy,
                    )
                    nc.scalar.dma_start(out[s, :, o0:o1], o_sb[:, o0:o1])

                # denominators (normalization happens on host).  The psum
                # copy rides ACT, not DVE: on DVE it waits for the ones
                # matmul (which runs after stage C) and head-of-line blocks
                # the NEXT slice's fold in the in-order DVE queue.
                psM = psm_pool.tile([1, _NB], f32, tag="psM")
                nc.tensor.matmul(psM[0:1, :], ones32[:, 0:1], e_kn[:, :])
                nc.scalar.activation(
                    den_all[:, _NB * s : _NB * s + _NB], psM[0:1, :],
                    mybir.ActivationFunctionType.Copy,
                )
            nc.scalar.dma_start(den[:, :], den_all[:])

    nc.compile()
    return nc


def _get_program():
    if "nc" not in _prog_cache:
        _prog_cache["nc"] = _build_program()
    return _prog_cache["nc"]


def _host_inputs(x, W_gate):
    bf16 = ml_dtypes.bfloat16
    x = np.asarray(x, dtype=np.float32)
    W = np.asarray(W_gate, dtype=np.float32)
    # column block p holds residue k'_p = (p//2) + 8*(p%2), interleaving the
    # two PE column groups so the DMA stream feeds them alternately
    perm = [(p // 2) + 8 * (p % 2) for p in range(16)]
    # wt2[d, 64p + 32j + g] = W_gate[g, (16j+k'_p)*128 + d]
    W4 = W.reshape(_K, 2, _NR, _D)  # [g, j, k', d]
    wt2_host = np.ascontiguousarray(
        W4.transpose(3, 2, 1, 0)[:, perm].reshape(_D, _NR * 64)
    ).astype(bf16)
    in_maps = []
    for core in range(_NC):
        xn = np.empty((_SL, 128, 32 * _D), dtype=bf16)
        xtk = np.empty((_SL, 128, _NR * _NM), dtype=bf16)
        for si in range(_SL):
            p = core * _SL + si
            b, h = p // _H, p % _H
            xs = x[b, :, h, :]  # [4096, 128]
            xn[si] = (
                xs.reshape(32, 128, _D).transpose(1, 0, 2).reshape(128, 32 * _D)
            ).astype(bf16)
            # xtk[d, 256p + m] = xs[16m + k'_p, d]
            xtk[si] = (
                xs.reshape(_NM, _NR, _D).transpose(2, 1, 0)[:, perm]
                .reshape(128, _NR * _NM)
            ).astype(bf16)
        in_maps.append({"xn": xn, "xtk": xtk, "wt2": wt2_host})
    return in_maps


def _assemble(results):
    out = np.empty((_B, _NB, _H, _D), dtype=np.float32)
    for core in range(_NC):
        o = np.asarray(results[core]["out"], dtype=np.float32)  # [SL, 128, 255]
        dn = np.asarray(results[core]["den"], dtype=np.float32).reshape(_SL, _NB)
        for si in range(_SL):
            p = core * _SL + si
            out[p // _H, :, p % _H, :] = o[si].T / dn[si][:, None]
    return out


def _install_trace_hooks():
    """Shim the axon NTFF profile hook (missing in this image) so
    run_bass_kernel_spmd(trace=True) can collect a HW profile, and neuter
    the artifact upload (zero-egress container)."""
    import contextlib
    import ctypes
    import types

    try:
        from antenv.axon_hooks import get_axon_ntff_profile_hook  # noqa: F401

        return
    except ImportError:
        pass

    lib = ctypes.CDLL("/opt/axon/libaxon_pjrt.so")
    if not hasattr(lib, "axon_start_nrt_profile"):
        return
    lib.axon_start_nrt_profile.argtypes = [
        ctypes.POINTER(ctypes.c_int64),
        ctypes.c_size_t,
    ]
    lib.axon_start_nrt_profile.restype = ctypes.c_int64
    lib.axon_stop_nrt_profile.argtypes = [ctypes.c_char_p]
    lib.axon_stop_nrt_profile.restype = ctypes.c_int64

    @contextlib.contextmanager
    def _hook(output_dir, device_ids):
        import jax

        jax.devices()
        if device_ids:
            ids = (ctypes.c_int64 * len(device_ids))(*device_ids)
            rc = lib.axon_start_nrt_profile(ids, len(device_ids))
        else:
            rc = lib.axon_start_nrt_profile(None, 0)
        if rc != 0:
            raise RuntimeError(f"axon_start_nrt_profile rc={rc}")
        try:
            yield
        finally:
            n = lib.axon_stop_nrt_profile(str(output_dir).encode())
            print(f"profile: {n} file(s) written to {output_dir}")

    mod = types.ModuleType("antenv.axon_hooks")
    mod.get_axon_ntff_profile_hook = lambda: _hook
    mod.set_axon_ntff_profile_hook = lambda h: None
    sys.modules["antenv.axon_hooks"] = mod

    from concourse import bass_utils as bu

    bu.upload_artifacts = lambda tmpdir: tmpdir


def run(x, W_gate, trace=False, **kw):
    from concourse.bass_utils import run_bass_kernel_spmd

    if trace:
        _install_trace_hooks()
    nc = _get_program()
    in_maps = _host_inputs(x, W_gate)
    res = run_bass_kernel_spmd(nc, in_maps, list(range(_NC)), trace=trace, **kw)
    return _assemble(res.results), res


def kernel(x, W_gate):
    out, _ = run(x, W_gate)
    return out
